# revision 9
# baseline (speedup 1.0000x reference)
"""Causal self-attention (B=4, L=2048, D=1024, H=16) on 8 Trainium2 NeuronCores.

Sharding: core c -> (batch b = c//2, head-group g = c%2 of 8 heads).
Each core computes qkv projection for its 8 heads, causal attention, and a
partial out-projection (its head-group's rows of W_out). The host sums the
two partials per batch and adds biases (exact: out-proj is linear and the
v-bias passes through softmax-weighted averaging).

v2 design (vs v1) — the kernel is PE-column-throughput bound, so:
  - x and W_q/k/v in bf16 (same PE rate as fp32r, half the DMA/SBUF):
    the full L=2048 stays resident, weights loaded once, DMAs batched.
  - Diagonal trimming: for j-tiles on the causal diagonal only the valid
    column range [o*128, 512) is matmul'd/exp'd (min width 256 dodges the
    fp32r small-free-size PE penalty; the [256,384) gap at o=3 is
    memset-zeroed so the AV accumulation sees zeros).
  - ACT (exp is ACT-only and nearly matches PE in the attention phase)
    does only exp + qk-projection PSUM evacuation; masks and the AV
    normalization run on DVE (+ Pool partition-broadcast, whose real
    per-op cost is too high for anything on the critical path).
  - es (exp scores) and vext in bf16: same PE rate, half SBUF.
  - Single 8-bank PSUM plan: tag s=[128,1024]x2 (scores + qk-proj),
    o0/o1=[65,512]x1 (AV accum), y=[128,512]x2 (v-proj + out-proj).
  - v-proj of l-tiles 12-15 deferred into the first attention q-block as
    PE filler; out-projection runs one q-block behind attention.

Attention layout (transpose-free), per head-pair (2 heads on 128 parts):
  S^T tile [128k x 512q] = kT_tile.T @ qT_block   (PE, K=64, tile_position)
  expS     = exp(S^T)  (ACT, PSUM->SBUF, bf16), causal band on diagonal
  O^T,sums [65 x 512q] += [V_tile | ones].T-form @ expS  (PE, K=128)
  O^T_norm = O^T * broadcast(1/sums)  -> directly the lhsT of out-proj
  Y tile   [128l x 512e] = sum_pairs O^T_pair.T @ Wo_pair
"""

import os
from contextlib import ExitStack

import numpy as np
import ml_dtypes

os.environ.setdefault("JAX_PLATFORMS", "")

import concourse.bass as bass
import concourse.mybir as mybir
import concourse.tile as tile
from concourse import bacc, bass_utils

F32 = mybir.dt.float32
F32R = mybir.dt.float32r
BF16 = mybir.dt.bfloat16
AF = mybir.ActivationFunctionType
NPBF16 = ml_dtypes.bfloat16

B, L, D, H = 4, 2048, 1024, 16
DK = D // H            # 64
G = 2                  # head groups (tensor parallel)
HPG = H // G           # 8 heads per group
GW = HPG * DK          # 512 columns per group
P = 128
CO = D // P            # 8 contraction tiles for projections
LT = L // P            # 16 l-tiles / k-tiles
QW = 512               # q-block width
QB = L // QW           # 4 q-blocks
NPAIR = HPG // 2       # 4 head-pairs per group (2 heads per 128 partitions)

_NC_CACHE: dict = {}


def build_nc(with_qk_bias: bool, repeat: int = 1):
    nc = bacc.Bacc("TRN2", target_bir_lowering=False, debug=False, num_devices=8)

    xt = nc.dram_tensor("xt", [D, L], BF16, kind="ExternalInput").ap()
    wq = nc.dram_tensor("wq", [D, GW], BF16, kind="ExternalInput").ap()
    wk = nc.dram_tensor("wk", [D, GW], BF16, kind="ExternalInput").ap()
    wv = nc.dram_tensor("wv", [D, GW], BF16, kind="ExternalInput").ap()
    wo = nc.dram_tensor("wo", [GW, D], F32, kind="ExternalInput").ap()
    mbh = nc.dram_tensor("mbh", [P, 4 * P], BF16, kind="ExternalInput").ap()
    if with_qk_bias:
        bq = nc.dram_tensor("bq", [P, NPAIR], F32, kind="ExternalInput").ap()
        bk = nc.dram_tensor("bk", [P, NPAIR], F32, kind="ExternalInput").ap()
    y = nc.dram_tensor("y", [L, D], F32, kind="ExternalOutput").ap()

    xt_r = xt.rearrange("(co p) l -> co p l", p=P)
    wq_r = wq.rearrange("(co p) c -> co p c", p=P)
    wk_r = wk.rearrange("(co p) c -> co p c", p=P)
    wv_r = wv.rearrange("(co p) c -> co p c", p=P)
    wo_r = wo.rearrange("(pr p) e -> pr p e", p=P)
    y_r = y.rearrange("(lt p) e -> lt p e", p=P)

    def mm(out, lhsT, rhs, start, stop, **kw):
        nc.tensor.matmul(out, lhsT, rhs, start=start, stop=stop, **kw)

    with tile.TileContext(nc) as tc, ExitStack() as ctx:
        constp = ctx.enter_context(tc.tile_pool(name="const", bufs=1))
        # causal band mask: mbh[k, i] = 1.0 iff i - 384 >= k (bf16)
        mbh_sb = constp.tile([P, 4 * P], BF16)
        nc.sync.dma_start(mbh_sb[:], mbh)
        # all-ones column (last col of the band mask)
        ones_sb = constp.tile([P, 1], BF16)
        nc.sync.dma_start(ones_sb[:], mbh[:, 4 * P - 1:4 * P])
        if with_qk_bias:
            bq_sb = constp.tile([P, NPAIR], F32)
            bk_sb = constp.tile([P, NPAIR], F32)
            nc.sync.dma_start(bq_sb[:], bq)
            nc.sync.dma_start(bk_sb[:], bk)

        qkp = ctx.enter_context(tc.tile_pool(name="qk", bufs=1))
        qT = qkp.tile([P, NPAIR, L], BF16)   # [d-in-pair, pair, l]
        kT = qkp.tile([P, NPAIR, L], BF16)
        vp = ctx.enter_context(tc.tile_pool(name="v", bufs=1))
        vext = vp.tile([P, LT, HPG, DK + 1], BF16)  # [l-in-tile, lt, head, d|ones]
        otp = ctx.enter_context(tc.tile_pool(name="ot", bufs=2))
        wop = ctx.enter_context(tc.tile_pool(name="wo", bufs=1))

        for _rep in range(repeat):
            _kernel_body(nc, tc, mm, with_qk_bias, locals())

    nc.compile()
    return nc


def _kernel_body(nc, tc, mm, with_qk_bias, env):
    qT, kT, vext, otp, wop = (env["qT"], env["kT"], env["vext"],
                              env["otp"], env["wop"])
    mbh_sb, ones_sb = env["mbh_sb"], env["ones_sb"]
    xt_r, wq_r, wk_r, wv_r, wo_r, y_r = (env["xt_r"], env["wq_r"], env["wk_r"],
                                         env["wv_r"], env["wo_r"], env["y_r"])
    bq_sb = env.get("bq_sb")
    bk_sb = env.get("bk_sb")

    with tc.tile_pool(name="xt", bufs=1) as xtp, \
         tc.tile_pool(name="w", bufs=2) as wp, \
         tc.tile_pool(name="es", bufs=3) as esp, \
         tc.tile_pool(name="rc", bufs=2) as rcp, \
         tc.tile_pool(name="yb", bufs=4) as ybp, \
         tc.tile_pool(name="ps", bufs=1, space="PSUM") as psp:

        # ---------------- input DMAs ----------------
        # batched (few dispatches — SP DGE setup is ~565ns each) and ordered
        # by first use: xt columns [0,1024) + wq feed the first proj tiles
        LH = L // 2
        CH = CO // 2
        xt_sb = xtp.tile([P, CO, L], BF16, tag="xt")
        wq_sb = wp.tile([P, CO, GW], BF16, tag="w")
        # first-needed chunks in co-pair granularity: few enough dispatches
        # (SP DGE setup ~565ns each) while still spreading across DMA queues
        for co in range(0, CO, 2):
            cs = slice(co, co + 2)
            nc.sync.dma_start(wq_sb[:, cs], wq_r[cs].transpose([1, 0, 2]))
            nc.sync.dma_start(xt_sb[:, cs, 0:LH],
                              xt_r[cs, :, 0:LH].transpose([1, 0, 2]))
        wk_sb = wp.tile([P, CO, GW], BF16, tag="w")
        for ch in range(2):
            cs = slice(ch * CH, (ch + 1) * CH)
            nc.sync.dma_start(xt_sb[:, cs, LH:L], xt_r[cs, :, LH:L].transpose([1, 0, 2]))
            nc.sync.dma_start(wk_sb[:, cs], wk_r[cs].transpose([1, 0, 2]))
        wv_sb = wp.tile([P, CO, GW], BF16, tag="w")
        for ch in range(2):
            cs = slice(ch * CH, (ch + 1) * CH)
            nc.sync.dma_start(wv_sb[:, cs], wv_r[cs].transpose([1, 0, 2]))
        wo_sb = wop.tile([P, NPAIR, D], F32R)
        nc.sync.dma_start(wo_sb[:], wo_r.transpose([1, 0, 2]).bitcast(F32R))

        # ---------------- phase 1: qkv projection ----------------
        for w_sb, dest, bias_sb in ((wq_sb, qT, bq_sb), (wk_sb, kT, bk_sb)):
            for lc in range(L // (2 * QW)):
                for pair in range(NPAIR):
                    pt = psp.tile([P, 2 * QW], F32, tag="s", bufs=2)
                    for half in range(2):
                        cb = lc * 2 * QW + half * QW
                        for co in range(CO):
                            mm(pt[:, half * QW:(half + 1) * QW],
                               w_sb[:, co, pair * P:(pair + 1) * P],
                               xt_sb[:, co, cb:cb + QW],
                               start=co == 0, stop=co == CO - 1)
                    dslice = dest[:, pair, lc * 2 * QW:(lc + 1) * 2 * QW]
                    if with_qk_bias:
                        nc.vector.tensor_scalar_add(
                            dslice, pt[:], bias_sb[:, pair:pair + 1])
                    else:
                        nc.scalar.copy(dslice, pt[:])

        def v_proj(lt):
            pv = psp.tile([P, QW], F32, tag="y", bufs=2, name="pv")
            for co in range(CO):
                mm(pv[:], xt_sb[:, co, lt * P:(lt + 1) * P], wv_sb[:, co],
                   start=co == 0, stop=co == CO - 1)
            nc.vector.tensor_copy(
                vext[:, lt, :, 0:DK],
                pv[:].rearrange("p (h d) -> p h d", h=HPG))

        # lt 12-15 are deferred into attention qb0's pair loop: they are only
        # needed from qb3 and give the first q-block its PE filler work
        for lt in range(12):
            v_proj(lt)
        # sums-trick ones column (written once, col DK: the AV psum rows are
        # then [0:64)=O^T (partition-aligned for DVE) and 64=sums)
        nc.vector.tensor_copy(
            vext[:, :, :, DK:DK + 1],
            ones_sb[:, None, None, :].to_broadcast((P, LT, HPG, 1)))

        # ------------- phase 2+3: attention + out-projection -------------
        # Out-projection runs one q-block behind attention (software pipeline)
        # so PE never stalls on the normalization chain.
        tri = mbh_sb[:, 3 * P:4 * P]  # [k, c']: 1 iff c' >= k

        def out_proj_part(oT, qb, i, split_dma=False):
            lt = 4 * qb + i
            yb = ybp.tile([P, D], F32, tag="yb")
            for eh in range(2):
                py = psp.tile([P, QW], F32, tag="y", bufs=2)
                for pair in range(NPAIR):
                    mm(py[:],
                       oT[:, pair, i * P:(i + 1) * P],
                       wo_sb[:, pair, eh * QW:(eh + 1) * QW],
                       start=pair == 0, stop=pair == NPAIR - 1)
                nc.vector.tensor_copy(yb[:, eh * QW:(eh + 1) * QW], py[:])
                if split_dma:
                    # final drain: ship each half as soon as it's copied
                    nc.sync.dma_start(y_r[lt, :, eh * QW:(eh + 1) * QW],
                                      yb[:, eh * QW:(eh + 1) * QW])
            if not split_dma:
                nc.sync.dma_start(y_r[lt], yb[:])

        prev = None
        for qb in range(QB):
            nj = 4 * qb + 4          # number of valid k-tiles
            oT = otp.tile([P, NPAIR, QW], F32R, tag="ot")
            for pair in range(NPAIR):
                po0 = psp.tile([DK + 1, QW], F32, tag="o0", name="po0")
                po1 = psp.tile([DK + 1, QW], F32, tag="o1", name="po1")
                for j in range(nj):
                    o = j - 4 * qb   # diagonal offset (>=0: on/above band)
                    # trimmed column range [lo, 512) (bf16 scores have no
                    # small-free-size PE penalty, so trim fully)
                    lo = 0 if o < 0 else o * P
                    ps2 = psp.tile([P, 2 * QW], F32, tag="s", bufs=2)
                    es2 = esp.tile([P, 2 * QW], BF16, tag="es")
                    mm(ps2[:, lo:QW],
                       kT[0:DK, pair, j * P:(j + 1) * P],
                       qT[0:DK, pair, qb * QW + lo:(qb + 1) * QW],
                       start=True, stop=True, tile_position=(0, 0))
                    mm(ps2[:, QW + lo:2 * QW],
                       kT[DK:P, pair, j * P:(j + 1) * P],
                       qT[DK:P, pair, qb * QW + lo:(qb + 1) * QW],
                       start=True, stop=True, tile_position=(64, 0))
                    pv2 = ps2.rearrange("p (u q) -> p u q", u=2)
                    ev2 = es2.rearrange("p (u q) -> p u q", u=2)
                    nc.scalar.activation(ev2[:, :, lo:QW], pv2[:, :, lo:QW],
                                         AF.Exp)
                    if o >= 0:
                        # triangular causal boundary on the diagonal slab
                        slab = ev2[:, :, o * P:(o + 1) * P]
                        nc.vector.tensor_mul(
                            slab, slab, tri[:, None, :].to_broadcast((P, 2, P)))
                    mm(po0[:, lo:QW], vext[:, j, 2 * pair, :],
                       es2[:, lo:QW],
                       start=j == 0, stop=j == nj - 1)
                    mm(po1[:, lo:QW], vext[:, j, 2 * pair + 1, :],
                       es2[:, QW + lo:2 * QW],
                       start=j == 0, stop=j == nj - 1)
                last = qb == QB - 1 and pair == NPAIR - 1
                for u, po in ((0, po0), (1, po1)):
                    # evacuate PSUM immediately (frees the bank for the next
                    # pair's AV): O^T rows + sums row (cross-copied to
                    # partition 0 for approx-recip) on DVE; Pool cannot
                    # read PSUM. The very last pair normalizes straight from
                    # PSUM instead — nothing needs its banks, and the shorter
                    # chain unblocks the final out-projection sooner.
                    if not last:
                        posb = rcp.tile([DK, QW], F32, tag="posb")
                        nc.vector.tensor_copy(posb[:], po[0:DK, :])
                    sm = rcp.tile([1, QW], F32, tag="sm")
                    nc.vector.tensor_copy(sm[:], po[DK:DK + 1, :])
                    rc = rcp.tile([1, QW], F32, tag="rc")
                    nc.vector.reciprocal_approx_fast(rc[:], sm[:])
                    rcb = rcp.tile([DK, QW], F32, tag="rcb")
                    nc.gpsimd.partition_broadcast(rcb[:], rc[:])
                    nc.vector.tensor_mul(
                        oT[u * DK:(u + 1) * DK, pair, :],
                        po[0:DK, :] if last else posb[:], rcb[:])
                # previous q-block's out-projection, interleaved per pair
                # (first q-block: the deferred v-projection tiles instead)
                if prev is not None:
                    out_proj_part(prev[0], prev[1], pair)
                else:
                    v_proj(12 + pair)
            prev = (oT, qb)
        for i in range(4):
            out_proj_part(prev[0], prev[1], i, split_dma=i == 3)


def _prep_inputs(x, W_qkv, b_qkv, W_out):
    """Per-core input maps. Core c -> batch c//2, head-group c%2."""
    x = np.asarray(x, dtype=np.float32)
    W_qkv = np.asarray(W_qkv, dtype=np.float32)
    b_qkv = np.asarray(b_qkv, dtype=np.float32)
    W_out = np.asarray(W_out, dtype=np.float32)

    scale = 1.0 / np.sqrt(DK)
    mbh = (np.arange(4 * P)[None, :] - 3 * P
           >= np.arange(P)[:, None]).astype(NPBF16)

    with_qk_bias = bool(np.any(b_qkv[:2 * D]))
    xts = [np.ascontiguousarray(x[b].T.astype(NPBF16)) for b in range(B)]
    wqs, wks, wvs = [], [], []
    for g in range(G):
        sl = slice(g * GW, (g + 1) * GW)
        wqs.append(np.ascontiguousarray(
            (W_qkv[:, g * GW:(g + 1) * GW] * scale).astype(NPBF16)))
        wks.append(np.ascontiguousarray(
            W_qkv[:, D + g * GW:D + (g + 1) * GW].astype(NPBF16)))
        wvs.append(np.ascontiguousarray(
            W_qkv[:, 2 * D + g * GW:2 * D + (g + 1) * GW].astype(NPBF16)))
    in_maps = []
    for c in range(8):
        b, g = c // 2, c % 2
        m = {
            "xt": xts[b],
            "wq": wqs[g],
            "wk": wks[g],
            "wv": wvs[g],
            "wo": np.ascontiguousarray(W_out[g * GW:(g + 1) * GW, :]),
            "mbh": mbh,
        }
        if with_qk_bias:
            m["bq"] = np.ascontiguousarray(
                b_qkv[g * GW:(g + 1) * GW].reshape(NPAIR, P).T) * scale
            m["bk"] = np.ascontiguousarray(
                b_qkv[D + g * GW:D + (g + 1) * GW].reshape(NPAIR, P).T)
        in_maps.append(m)
    return in_maps, with_qk_bias


def kernel(x, W_qkv, b_qkv, W_out, b_out):
    in_maps, with_qk_bias = _prep_inputs(x, W_qkv, b_qkv, W_out)

    key = ("nc", with_qk_bias)
    if key not in _NC_CACHE:
        _NC_CACHE[key] = build_nc(with_qk_bias)
    nc = _NC_CACHE[key]

    res = bass_utils.run_bass_kernel_spmd(nc, in_maps, core_ids=list(range(8)))
    parts = [r["y"] for r in res.results]

    b_qkv = np.asarray(b_qkv, dtype=np.float32)
    W_out_np = np.asarray(W_out, dtype=np.float32)
    # v-bias passes through attention (rows of attn sum to 1) and out-proj is
    # linear: contribution = b_v @ W_out; b_out adds directly.
    corr = (b_qkv[2 * D:3 * D] @ W_out_np
            + np.asarray(b_out, dtype=np.float32)).astype(np.float32)

    out = np.empty((B, L, D), dtype=np.float32)
    for b in range(B):
        out[b] = parts[2 * b] + parts[2 * b + 1] + corr
    return out


# revision 10
# speedup vs baseline: 1.0821x; 1.0821x over previous
"""Causal self-attention (B=4, L=2048, D=1024, H=16) on 8 Trainium2 NeuronCores.

Sharding: core c -> (batch b = c//2, head-group g = c%2 of 8 heads).
Each core computes qkv projection for its 8 heads, causal attention, and a
partial out-projection (its head-group's rows of W_out). The host sums the
two partials per batch and adds biases (exact: out-proj is linear and the
v-bias passes through softmax-weighted averaging).

Design (the kernel is PE-column-throughput bound, so):
  - x, W_q/k/v, qT/kT, es, vext all in bf16 (same PE rate as fp32r, half
    the DMA/SBUF): the full L=2048 stays resident, weights loaded once,
    DMAs batched. bf16 also has no small-free-size matmul penalty, so
    diagonal j-tiles are trimmed to exactly their valid column range
    [o*128, 512) in the scores matmul, exp, and AV matmul alike.
  - ACT (exp is ACT-only on trn2 — DVE shift ops return 0, so no custom
    exp there — and nearly matches PE in the attention phase) does only
    exp + qk-projection PSUM evacuation; masks and the AV normalization
    run on DVE (+ Pool partition-broadcast); Pool/ACT/DVE placements are
    A/B-validated, every alternative measured worse.
  - Single 8-bank PSUM plan: tag s=[128,1024]x2 (scores + qk-proj),
    o0/o1=[65,512]x1 (AV accum), y=[128,512]x2 (v-proj + out-proj).
  - v-proj of l-tiles 12-15 deferred into the first attention q-block as
    PE filler; out-projection runs one q-block behind attention; the last
    pair normalizes straight from PSUM and the final tile's output DMA is
    split per half to shorten the drain.

Attention layout (transpose-free), per head-pair (2 heads on 128 parts):
  S^T tile [128k x 512q] = kT_tile.T @ qT_block   (PE, K=64, tile_position)
  expS     = exp(S^T)  (ACT, PSUM->SBUF, bf16), causal band on diagonal
  O^T,sums [65 x 512q] += [V_tile | ones].T-form @ expS  (PE, K=128)
  O^T_norm = O^T * broadcast(1/sums)  -> directly the lhsT of out-proj
  Y tile   [128l x 512e] = sum_pairs O^T_pair.T @ Wo_pair
"""

import os
from contextlib import ExitStack

import numpy as np
import ml_dtypes

os.environ.setdefault("JAX_PLATFORMS", "")

import concourse.bass as bass
import concourse.mybir as mybir
import concourse.tile as tile
from concourse import bacc, bass_utils

F32 = mybir.dt.float32
F32R = mybir.dt.float32r
BF16 = mybir.dt.bfloat16
AF = mybir.ActivationFunctionType
NPBF16 = ml_dtypes.bfloat16

B, L, D, H = 4, 2048, 1024, 16
DK = D // H            # 64
G = 2                  # head groups (tensor parallel)
HPG = H // G           # 8 heads per group
GW = HPG * DK          # 512 columns per group
P = 128
CO = D // P            # 8 contraction tiles for projections
LT = L // P            # 16 l-tiles / k-tiles
QW = 512               # q-block width
QB = L // QW           # 4 q-blocks
NPAIR = HPG // 2       # 4 head-pairs per group (2 heads per 128 partitions)

_NC_CACHE: dict = {}


def build_nc(with_qk_bias: bool, repeat: int = 1):
    nc = bacc.Bacc("TRN2", target_bir_lowering=False, debug=False, num_devices=8)

    xt = nc.dram_tensor("xt", [D, L], BF16, kind="ExternalInput").ap()
    wq = nc.dram_tensor("wq", [D, GW], BF16, kind="ExternalInput").ap()
    wk = nc.dram_tensor("wk", [D, GW], BF16, kind="ExternalInput").ap()
    wv = nc.dram_tensor("wv", [D, GW], BF16, kind="ExternalInput").ap()
    wo = nc.dram_tensor("wo", [GW, D], F32, kind="ExternalInput").ap()
    mbh = nc.dram_tensor("mbh", [P, 4 * P], BF16, kind="ExternalInput").ap()
    if with_qk_bias:
        bq = nc.dram_tensor("bq", [P, NPAIR], F32, kind="ExternalInput").ap()
        bk = nc.dram_tensor("bk", [P, NPAIR], F32, kind="ExternalInput").ap()
    y = nc.dram_tensor("y", [L, D], F32, kind="ExternalOutput").ap()

    xt_r = xt.rearrange("(co p) l -> co p l", p=P)
    wq_r = wq.rearrange("(co p) c -> co p c", p=P)
    wk_r = wk.rearrange("(co p) c -> co p c", p=P)
    wv_r = wv.rearrange("(co p) c -> co p c", p=P)
    wo_r = wo.rearrange("(pr p) e -> pr p e", p=P)
    y_r = y.rearrange("(lt p) e -> lt p e", p=P)

    def mm(out, lhsT, rhs, start, stop, **kw):
        nc.tensor.matmul(out, lhsT, rhs, start=start, stop=stop, **kw)

    with tile.TileContext(nc) as tc, ExitStack() as ctx:
        constp = ctx.enter_context(tc.tile_pool(name="const", bufs=1))
        # causal band mask: mbh[k, i] = 1.0 iff i - 384 >= k (bf16)
        mbh_sb = constp.tile([P, 4 * P], BF16)
        nc.sync.dma_start(mbh_sb[:], mbh)
        # all-ones column (last col of the band mask)
        ones_sb = constp.tile([P, 1], BF16)
        nc.sync.dma_start(ones_sb[:], mbh[:, 4 * P - 1:4 * P])
        if with_qk_bias:
            bq_sb = constp.tile([P, NPAIR], F32)
            bk_sb = constp.tile([P, NPAIR], F32)
            nc.sync.dma_start(bq_sb[:], bq)
            nc.sync.dma_start(bk_sb[:], bk)

        qkp = ctx.enter_context(tc.tile_pool(name="qk", bufs=1))
        qT = qkp.tile([P, NPAIR, L], BF16)   # [d-in-pair, pair, l]
        kT = qkp.tile([P, NPAIR, L], BF16)
        vp = ctx.enter_context(tc.tile_pool(name="v", bufs=1))
        vext = vp.tile([P, LT, HPG, DK + 1], BF16)  # [l-in-tile, lt, head, d|ones]
        otp = ctx.enter_context(tc.tile_pool(name="ot", bufs=2))
        wop = ctx.enter_context(tc.tile_pool(name="wo", bufs=1))

        for _rep in range(repeat):
            _kernel_body(nc, tc, mm, with_qk_bias, locals())

    nc.compile()
    return nc


def _kernel_body(nc, tc, mm, with_qk_bias, env):
    qT, kT, vext, otp, wop = (env["qT"], env["kT"], env["vext"],
                              env["otp"], env["wop"])
    mbh_sb, ones_sb = env["mbh_sb"], env["ones_sb"]
    xt_r, wq_r, wk_r, wv_r, wo_r, y_r = (env["xt_r"], env["wq_r"], env["wk_r"],
                                         env["wv_r"], env["wo_r"], env["y_r"])
    bq_sb = env.get("bq_sb")
    bk_sb = env.get("bk_sb")

    with tc.tile_pool(name="xt", bufs=1) as xtp, \
         tc.tile_pool(name="w", bufs=2) as wp, \
         tc.tile_pool(name="es", bufs=3) as esp, \
         tc.tile_pool(name="rc", bufs=2) as rcp, \
         tc.tile_pool(name="yb", bufs=4) as ybp, \
         tc.tile_pool(name="ps", bufs=1, space="PSUM") as psp:

        # ---------------- input DMAs ----------------
        # batched (few dispatches — SP DGE setup is ~565ns each) and ordered
        # by first use: xt columns [0,1024) + wq feed the first proj tiles
        LH = L // 2
        CH = CO // 2
        xt_sb = xtp.tile([P, CO, L], BF16, tag="xt")
        wq_sb = wp.tile([P, CO, GW], BF16, tag="w")
        # first-needed chunks in co-pair granularity: few enough dispatches
        # (SP DGE setup ~565ns each) while still spreading across DMA queues
        for co in range(0, CO, 2):
            cs = slice(co, co + 2)
            nc.sync.dma_start(wq_sb[:, cs], wq_r[cs].transpose([1, 0, 2]))
            nc.sync.dma_start(xt_sb[:, cs, 0:LH],
                              xt_r[cs, :, 0:LH].transpose([1, 0, 2]))
        wk_sb = wp.tile([P, CO, GW], BF16, tag="w")
        for ch in range(2):
            cs = slice(ch * CH, (ch + 1) * CH)
            nc.sync.dma_start(xt_sb[:, cs, LH:L], xt_r[cs, :, LH:L].transpose([1, 0, 2]))
            nc.sync.dma_start(wk_sb[:, cs], wk_r[cs].transpose([1, 0, 2]))
        wv_sb = wp.tile([P, CO, GW], BF16, tag="w")
        for ch in range(2):
            cs = slice(ch * CH, (ch + 1) * CH)
            nc.sync.dma_start(wv_sb[:, cs], wv_r[cs].transpose([1, 0, 2]))
        wo_sb = wop.tile([P, NPAIR, D], F32R)
        nc.sync.dma_start(wo_sb[:], wo_r.transpose([1, 0, 2]).bitcast(F32R))

        # ---------------- phase 1: qkv projection ----------------
        for w_sb, dest, bias_sb in ((wq_sb, qT, bq_sb), (wk_sb, kT, bk_sb)):
            for lc in range(L // (2 * QW)):
                for pair in range(NPAIR):
                    pt = psp.tile([P, 2 * QW], F32, tag="s", bufs=2)
                    for half in range(2):
                        cb = lc * 2 * QW + half * QW
                        for co in range(CO):
                            mm(pt[:, half * QW:(half + 1) * QW],
                               w_sb[:, co, pair * P:(pair + 1) * P],
                               xt_sb[:, co, cb:cb + QW],
                               start=co == 0, stop=co == CO - 1)
                    dslice = dest[:, pair, lc * 2 * QW:(lc + 1) * 2 * QW]
                    if with_qk_bias:
                        nc.vector.tensor_scalar_add(
                            dslice, pt[:], bias_sb[:, pair:pair + 1])
                    else:
                        nc.scalar.copy(dslice, pt[:])

        def v_proj(lt):
            pv = psp.tile([P, QW], F32, tag="y", bufs=2, name="pv")
            for co in range(CO):
                mm(pv[:], xt_sb[:, co, lt * P:(lt + 1) * P], wv_sb[:, co],
                   start=co == 0, stop=co == CO - 1)
            nc.vector.tensor_copy(
                vext[:, lt, :, 0:DK],
                pv[:].rearrange("p (h d) -> p h d", h=HPG))

        # lt 12-15 are deferred into attention qb0's pair loop: they are only
        # needed from qb3 and give the first q-block its PE filler work
        for lt in range(12):
            v_proj(lt)
        # sums-trick ones column (written once, col DK: the AV psum rows are
        # then [0:64)=O^T (partition-aligned for DVE) and 64=sums)
        nc.vector.tensor_copy(
            vext[:, :, :, DK:DK + 1],
            ones_sb[:, None, None, :].to_broadcast((P, LT, HPG, 1)))

        # ------------- phase 2+3: attention + out-projection -------------
        # Out-projection runs one q-block behind attention (software pipeline)
        # so PE never stalls on the normalization chain.
        tri = mbh_sb[:, 3 * P:4 * P]  # [k, c']: 1 iff c' >= k

        def out_proj_part(oT, qb, i, split_dma=False):
            lt = 4 * qb + i
            yb = ybp.tile([P, D], F32, tag="yb")
            for eh in range(2):
                py = psp.tile([P, QW], F32, tag="y", bufs=2)
                for pair in range(NPAIR):
                    mm(py[:],
                       oT[:, pair, i * P:(i + 1) * P],
                       wo_sb[:, pair, eh * QW:(eh + 1) * QW],
                       start=pair == 0, stop=pair == NPAIR - 1)
                nc.vector.tensor_copy(yb[:, eh * QW:(eh + 1) * QW], py[:])
                if split_dma:
                    # final drain: ship each half as soon as it's copied
                    nc.sync.dma_start(y_r[lt, :, eh * QW:(eh + 1) * QW],
                                      yb[:, eh * QW:(eh + 1) * QW])
            if not split_dma:
                nc.sync.dma_start(y_r[lt], yb[:])

        prev = None
        for qb in range(QB):
            nj = 4 * qb + 4          # number of valid k-tiles
            oT = otp.tile([P, NPAIR, QW], F32R, tag="ot")
            for pair in range(NPAIR):
                po0 = psp.tile([DK + 1, QW], F32, tag="o0", name="po0")
                po1 = psp.tile([DK + 1, QW], F32, tag="o1", name="po1")
                for j in range(nj):
                    o = j - 4 * qb   # diagonal offset (>=0: on/above band)
                    # trimmed column range [lo, 512) (bf16 scores have no
                    # small-free-size PE penalty, so trim fully)
                    lo = 0 if o < 0 else o * P
                    ps2 = psp.tile([P, 2 * QW], F32, tag="s", bufs=2)
                    es2 = esp.tile([P, 2 * QW], BF16, tag="es")
                    mm(ps2[:, lo:QW],
                       kT[0:DK, pair, j * P:(j + 1) * P],
                       qT[0:DK, pair, qb * QW + lo:(qb + 1) * QW],
                       start=True, stop=True, tile_position=(0, 0))
                    mm(ps2[:, QW + lo:2 * QW],
                       kT[DK:P, pair, j * P:(j + 1) * P],
                       qT[DK:P, pair, qb * QW + lo:(qb + 1) * QW],
                       start=True, stop=True, tile_position=(64, 0))
                    pv2 = ps2.rearrange("p (u q) -> p u q", u=2)
                    ev2 = es2.rearrange("p (u q) -> p u q", u=2)
                    nc.scalar.activation(ev2[:, :, lo:QW], pv2[:, :, lo:QW],
                                         AF.Exp)
                    if o >= 0:
                        # triangular causal boundary on the diagonal slab
                        slab = ev2[:, :, o * P:(o + 1) * P]
                        nc.vector.tensor_mul(
                            slab, slab, tri[:, None, :].to_broadcast((P, 2, P)))
                    mm(po0[:, lo:QW], vext[:, j, 2 * pair, :],
                       es2[:, lo:QW],
                       start=j == 0, stop=j == nj - 1)
                    mm(po1[:, lo:QW], vext[:, j, 2 * pair + 1, :],
                       es2[:, QW + lo:2 * QW],
                       start=j == 0, stop=j == nj - 1)
                last = qb == QB - 1 and pair == NPAIR - 1
                for u, po in ((0, po0), (1, po1)):
                    # evacuate PSUM immediately (frees the bank for the next
                    # pair's AV): O^T rows + sums row (cross-copied to
                    # partition 0 for approx-recip) on DVE; Pool cannot
                    # read PSUM. The very last pair normalizes straight from
                    # PSUM instead — nothing needs its banks, and the shorter
                    # chain unblocks the final out-projection sooner.
                    if not last:
                        posb = rcp.tile([DK, QW], F32, tag="posb")
                        nc.vector.tensor_copy(posb[:], po[0:DK, :])
                    sm = rcp.tile([1, QW], F32, tag="sm")
                    nc.vector.tensor_copy(sm[:], po[DK:DK + 1, :])
                    rc = rcp.tile([1, QW], F32, tag="rc")
                    nc.vector.reciprocal_approx_fast(rc[:], sm[:])
                    rcb = rcp.tile([DK, QW], F32, tag="rcb")
                    nc.gpsimd.partition_broadcast(rcb[:], rc[:])
                    nc.vector.tensor_mul(
                        oT[u * DK:(u + 1) * DK, pair, :],
                        po[0:DK, :] if last else posb[:], rcb[:])
                # previous q-block's out-projection, interleaved per pair
                # (first q-block: the deferred v-projection tiles instead)
                if prev is not None:
                    out_proj_part(prev[0], prev[1], pair)
                else:
                    v_proj(12 + pair)
            prev = (oT, qb)
        for i in range(4):
            out_proj_part(prev[0], prev[1], i, split_dma=i == 3)


def _prep_inputs(x, W_qkv, b_qkv, W_out):
    """Per-core input maps. Core c -> batch c//2, head-group c%2."""
    x = np.asarray(x, dtype=np.float32)
    W_qkv = np.asarray(W_qkv, dtype=np.float32)
    b_qkv = np.asarray(b_qkv, dtype=np.float32)
    W_out = np.asarray(W_out, dtype=np.float32)

    scale = 1.0 / np.sqrt(DK)
    mbh = (np.arange(4 * P)[None, :] - 3 * P
           >= np.arange(P)[:, None]).astype(NPBF16)

    with_qk_bias = bool(np.any(b_qkv[:2 * D]))
    xts = [np.ascontiguousarray(x[b].T.astype(NPBF16)) for b in range(B)]
    wqs, wks, wvs = [], [], []
    for g in range(G):
        sl = slice(g * GW, (g + 1) * GW)
        wqs.append(np.ascontiguousarray(
            (W_qkv[:, g * GW:(g + 1) * GW] * scale).astype(NPBF16)))
        wks.append(np.ascontiguousarray(
            W_qkv[:, D + g * GW:D + (g + 1) * GW].astype(NPBF16)))
        wvs.append(np.ascontiguousarray(
            W_qkv[:, 2 * D + g * GW:2 * D + (g + 1) * GW].astype(NPBF16)))
    in_maps = []
    for c in range(8):
        b, g = c // 2, c % 2
        m = {
            "xt": xts[b],
            "wq": wqs[g],
            "wk": wks[g],
            "wv": wvs[g],
            "wo": np.ascontiguousarray(W_out[g * GW:(g + 1) * GW, :]),
            "mbh": mbh,
        }
        if with_qk_bias:
            m["bq"] = np.ascontiguousarray(
                b_qkv[g * GW:(g + 1) * GW].reshape(NPAIR, P).T) * scale
            m["bk"] = np.ascontiguousarray(
                b_qkv[D + g * GW:D + (g + 1) * GW].reshape(NPAIR, P).T)
        in_maps.append(m)
    return in_maps, with_qk_bias


def kernel(x, W_qkv, b_qkv, W_out, b_out):
    in_maps, with_qk_bias = _prep_inputs(x, W_qkv, b_qkv, W_out)

    key = ("nc", with_qk_bias)
    if key not in _NC_CACHE:
        _NC_CACHE[key] = build_nc(with_qk_bias)
    nc = _NC_CACHE[key]

    res = bass_utils.run_bass_kernel_spmd(nc, in_maps, core_ids=list(range(8)))
    parts = [r["y"] for r in res.results]

    b_qkv = np.asarray(b_qkv, dtype=np.float32)
    W_out_np = np.asarray(W_out, dtype=np.float32)
    # v-bias passes through attention (rows of attn sum to 1) and out-proj is
    # linear: contribution = b_v @ W_out; b_out adds directly.
    corr = (b_qkv[2 * D:3 * D] @ W_out_np
            + np.asarray(b_out, dtype=np.float32)).astype(np.float32)

    out = np.empty((B, L, D), dtype=np.float32)
    for b in range(B):
        out[b] = parts[2 * b] + parts[2 * b + 1] + corr
    return out
